# revision 7
# baseline (speedup 1.0000x reference)
"""Trainium2 Bass kernel for nn_DynamicSparseConv.

Model (per sample):
    y  = mean(x, HW)                        [C]
    h  = gelu(y @ w1.T)                     [MID]
    w  = softmax((h @ w2.T).reshape(C, 9))  per-channel 3x3 kernels
    out = depthwise3x3(x, w) + x

Sharding: pure data parallel, batch 32 -> 4 samples on each of 8 cores.

Per-core design (per (sample b, channel-block cb of 128)):
  - x arrives f32 [128, 4096]; one ACT pass per half does the f32->bf16
    cast INTO a zero-padded pitch-66 tile xc ([66 rows x 66 cols], one
    zero col on each side of every row, zero rows top/bottom) AND the
    channel sums via accum_out.  The pitch-66 layout makes every 3x3 tap
    a clean 2D view [[66,nr],[1,64]] -- no horizontal wrap-around
    garbage, so no edge fixups at all.
  - tiny MLP on PE (wgen), tanh-gelu, softmax: as before.
  - conv: 6 taps on PE as diagonal-weighted matmuls into PSUM (center
    tap's diag holds w_center+1, fusing the residual); ACT drains each
    PSUM chunk to the f16 output tile; DVE applies the remaining 3 taps
    as TS-mul(4x) + TT-add(2x) pairs on pitch-66 views.  The last
    sample runs all 9 taps on PE so the kernel tail is just drain+DMA.
  - output is written f16 (half the DMA bytes); the host upcasts.
"""

import numpy as np
from contextlib import ExitStack

import concourse.bass as bass
import concourse.tile as tile
from concourse import mybir
from concourse._compat import with_exitstack
from concourse.masks import make_identity
from concourse.bass_utils import run_bass_kernel_spmd

F32 = mybir.dt.float32
F16 = mybir.dt.float16
BF16 = mybir.dt.bfloat16
AL = mybir.AluOpType
AF = mybir.ActivationFunctionType

B, C, H, W = 32, 256, 64, 64
MID = 32
NCORES = 8
BPC = B // NCORES          # samples per core
P = 128
CB = C // P                # channel blocks
FREE = H * W               # 4096
NCHUNK = 8                 # PSUM chunks per tile (8 output rows each)
RPC = H // NCHUNK          # rows per chunk
CH = RPC * W               # 512 elements per PSUM chunk

PITCH = W + 2              # 66
XC = PITCH * (H + 2)       # 4356: rows -1..64, cols -1..64

# tap -> smw column
def _tcol(r, s):
    return (r + 1) * 3 + (s + 1)

# taps: center rides PE with weight w+1 (residual fused)
PE_TAPS = [(0, 0), (-1, -1), (-1, 1), (1, -1), (1, 1), (0, -1)]
DVE_TAPS = [(-1, 0), (1, 0), (0, 1)]

SQRT_2_OVER_PI = 0.7978845608028654
GELU_C = 0.044715


def _v66(t, row0, col, nr, inner=W):
    """2D view of the pitch-66 tile: rows row0..row0+nr-1 (x row index,
    -1-based pad space), starting at x col `col` (-1..64)."""
    return bass.AP(
        tensor=t.tensor,
        offset=t.offset + (row0 + 1) * PITCH + (col + 1),
        ap=[list(t.ap[0]), [PITCH, nr], [1, inner]],
    )


@with_exitstack
def _build_body(ctx: ExitStack, tc: "tile.TileContext", x, w1t, w2r, out):
    nc = tc.nc

    consts = ctx.enter_context(tc.tile_pool(name="consts", bufs=1))
    xspool = ctx.enter_context(tc.tile_pool(name="xspool", bufs=4))
    xcpool = ctx.enter_context(tc.tile_pool(name="xcpool", bufs=5))
    opool = ctx.enter_context(tc.tile_pool(name="opool", bufs=4))
    tpool = ctx.enter_context(tc.tile_pool(name="tpool", bufs=3))
    mpool = ctx.enter_context(tc.tile_pool(name="mpool", bufs=4))
    dpool = ctx.enter_context(tc.tile_pool(name="dpool", bufs=4 * 10))
    cpsum = ctx.enter_context(tc.tile_pool(name="cpsum", bufs=6, space="PSUM"))
    spsum = ctx.enter_context(tc.tile_pool(name="spsum", bufs=2, space="PSUM"))

    NSPLIT = 2
    HROWS = H // NSPLIT
    st = {}

    def load(b):
        """DMA f32 halves; gp memsets zero the pad cells of xc."""
        xss, xcs, dmas = [], [], []
        sums = mpool.tile([P, CB * NSPLIT], F32, name=f"sums{b}", tag="sums")
        for cb in range(CB):
            xs = xspool.tile([P, FREE], BF16, name=f"xs{b}_{cb}", tag="xs")
            xc = xcpool.tile([P, XC], BF16, name=f"xc{b}_{cb}", tag="xc")
            # pads: top row (-1), bottom row (64), and the col-pad pairs
            # (col 64 of row r, col -1 of row r+1) which are adjacent.
            nc.gpsimd.memset(xc[:, 0:PITCH], 0.0)
            nc.gpsimd.memset(xc[:, XC - PITCH:XC], 0.0)
            pairs = bass.AP(
                tensor=xc.tensor, offset=xc.offset + PITCH - 1,
                ap=[list(xc.ap[0]), [PITCH, H + 1], [1, 2]],
            )
            nc.gpsimd.memset(pairs, 0.0)
            xsrc = x[b, cb * P:(cb + 1) * P].rearrange("c h w -> c (h w)")
            for j in range(NSPLIT):
                sl = slice(j * HROWS * W, (j + 1) * HROWS * W)
                dmas.append(nc.sync.dma_start(out=xs[:, sl], in_=xsrc[:, sl]))
                scol = sums[:, cb * NSPLIT + j:cb * NSPLIT + j + 1]
                if b == 0 and cb == 0:
                    # head: ACT is choked on casts; DVE (idle) computes the
                    # sums in parallel, the cast runs without accum
                    nc.vector.tensor_reduce(
                        out=scol, in_=xs[:, sl], op=AL.add,
                        axis=mybir.AxisListType.X)
                    nc.scalar.activation(
                        out=_v66(xc, j * HROWS, 0, HROWS),
                        in_=xs[:, sl], func=AF.Copy)
                else:
                    nc.scalar.activation(
                        out=_v66(xc, j * HROWS, 0, HROWS),
                        in_=xs[:, sl], func=AF.Copy, accum_out=scol)
            xss.append(xs)
            xcs.append(xc)
        st[b] = {"xss": xss, "xcs": xcs, "sums": sums, "dmas": dmas}

    def prep(b, weights):
        """MLP -> softmax tap weights -> diagonal matrices for sample b."""
        w1t_sb, w2r_sb, ident = weights
        sums = st[b]["sums"]
        ncols = CB * NSPLIT
        sums_bf = mpool.tile([P, ncols], BF16, name=f"sums_bf{b}", tag="sums_bf")
        # fold the 1/(H*W) of the mean in here
        nc.vector.tensor_scalar_mul(sums_bf, sums, 1.0 / FREE)
        hps = spsum.tile([P, 9], F32, name=f"hps{b}", tag="sps")
        for j in range(ncols):
            nc.tensor.matmul(
                hps[:, 0:1], lhsT=w1t_sb[:, j // NSPLIT, :],
                rhs=sums_bf[:, j:j + 1],
                start=(j == 0), stop=(j == ncols - 1),
            )
        # tanh-gelu (exact to ~1e-9 for |u|<0.05); chain stays on DVE except
        # the tanh itself so there are few cross-engine hops and no ACT
        # table swaps (Tanh+Exp share a table set, Gelu does not).
        u = mpool.tile([P, 1], F32, name=f"u{b}", tag="u")
        nc.vector.tensor_copy(u, hps[:, 0:1])
        sq = mpool.tile([P, 1], F32, name=f"sq{b}", tag="sq")
        nc.vector.tensor_mul(sq, u, u)
        c1 = mpool.tile([P, 1], F32, name=f"c1{b}", tag="c1")
        nc.vector.tensor_scalar(
            out=c1, in0=sq, scalar1=GELU_C, scalar2=1.0, op0=AL.mult, op1=AL.add,
        )
        arg = mpool.tile([P, 1], F32, name=f"arg{b}", tag="arg")
        nc.vector.tensor_mul(arg, u, c1)
        th = mpool.tile([P, 1], F32, name=f"th{b}", tag="th")
        nc.scalar.activation(th, arg, AF.Tanh, scale=SQRT_2_OVER_PI)
        g4 = mpool.tile([P, 1], BF16, name=f"g4{b}", tag="g4")
        nc.vector.tensor_scalar(
            out=g4, in0=th, scalar1=1.0, scalar2=u, op0=AL.add, op1=AL.mult,
        )

        # 18 wgen matmuls in the two 64-row PE tile groups
        wgs = [spsum.tile([P, 9], F32, name=f"wg{b}_{cb}", tag="sps")
               for cb in range(CB)]
        for t in range(9):
            for cb in range(CB):
                rg = 64 * cb
                nc.tensor.matmul(
                    wgs[cb][:, t:t + 1],
                    lhsT=w2r_sb[rg:rg + MID, t, :],
                    rhs=g4[rg:rg + MID, 0:1],
                    start=True, stop=True,
                    tile_position=(rg, 0),
                )

        st[b]["smw"] = [None] * CB
        st[b]["diags"] = [None] * CB
        last = b == BPC - 1
        cbs = [1, 0] if b == 0 else list(range(CB))
        for cb in cbs:
            pe_taps = PE_TAPS + (DVE_TAPS if (last and cb == CB - 1) else [])
            ew = mpool.tile([P, 9], F32, name=f"ew{b}_{cb}", tag="ew")
            den = mpool.tile([P, 1], F32, name=f"den{b}_{cb}", tag="den")
            nc.scalar.activation(ew, wgs[cb], AF.Exp, accum_out=den)
            rden = mpool.tile([P, 1], F32, name=f"rden{b}_{cb}", tag="rden")
            nc.vector.reciprocal(rden, den)
            smw = mpool.tile([P, 9], F32, name=f"smw{b}_{cb}", tag="smw")
            nc.vector.tensor_scalar_mul(smw, ew, rden)
            # center diag coefficient: w_center + 1 (residual fused)
            wc1 = mpool.tile([P, 1], F32, name=f"wc1{b}_{cb}", tag="wc1")
            nc.vector.tensor_scalar_add(wc1, smw[:, 4:5], 1.0)

            diags = {}
            for k, (r, s) in enumerate(pe_taps):
                tcol = _tcol(r, s)
                dg = dpool.tile([P, P], BF16, name=f"dg{b}_{cb}_{tcol}", tag="dg")
                sca = wc1 if (r, s) == (0, 0) else smw[:, tcol:tcol + 1]
                # sample 0: all diags on DVE (ACT is the head serializer);
                # later samples alternate ACT/DVE
                if b > 0 and k % 2 == 0:
                    nc.scalar.mul(dg, ident, sca)
                else:
                    nc.vector.tensor_scalar_mul(dg, ident, sca)
                diags[(r, s)] = dg
            st[b]["smw"][cb] = smw
            st[b]["diags"][cb] = diags

    def conv(b):
        """Depthwise conv: PE taps -> PSUM, ACT drain -> f16 ot, DVE taps
        accumulate on pitch-66 views, then DMA out per half."""
        last = b == BPC - 1
        cbs = [1, 0] if b == 0 else list(range(CB))
        for cb in cbs:
            pe_taps = PE_TAPS + (DVE_TAPS if (last and cb == CB - 1) else [])
            xc = st[b]["xcs"][cb]
            smw = st[b]["smw"][cb]
            diags = st[b]["diags"][cb]
            ot = opool.tile([P, FREE], F16, name=f"ot{b}_{cb}", tag="ot")
            for q in range(NCHUNK):
                ps = cpsum.tile([P, CH], F32, name=f"ps{b}_{cb}_{q}", tag="ps")
                for i, (r, s) in enumerate(pe_taps):
                    nc.tensor.matmul(
                        ps,
                        lhsT=diags[(r, s)],
                        rhs=_v66(xc, q * RPC + r, s, RPC),
                        start=(i == 0), stop=(i == len(pe_taps) - 1),
                    )
                # drain PSUM chunk into the f16 output tile: DVE early in
                # the pipeline (ACT is cast-choked there), ACT later (DVE
                # becomes the busier engine in steady state)
                if b == 0 or (b == 1 and q >= 6):
                    nc.vector.tensor_copy(ot[:, q * CH:(q + 1) * CH], ps)
                else:
                    nc.scalar.activation(
                        out=ot[:, q * CH:(q + 1) * CH], in_=ps, func=AF.Copy)

            # finer pieces late in the pipeline shorten the serial tail
            npiece = 4 if (last and cb == 0) else 2
            rows = H // npiece
            for piece in range(npiece):
                h0 = piece * rows
                osl = slice(h0 * W, (h0 + rows) * W)
                if not (last and cb == CB - 1):
                    tmp = tpool.tile([P, rows * W], BF16,
                                     name=f"tm{b}_{cb}_{piece}", tag="tm")
                    for (r, s) in DVE_TAPS:
                        tcol = _tcol(r, s)
                        nc.vector.tensor_scalar_mul(
                            tmp, _v66(xc, h0 + r, s, rows),
                            smw[:, tcol:tcol + 1])
                        nc.vector.tensor_tensor(
                            out=ot[:, osl], in0=ot[:, osl], in1=tmp, op=AL.add)
                eng = nc.sync if (cb + piece) % 2 == 0 else nc.gpsimd
                eng.dma_start(
                    out=out[b, cb * P:(cb + 1) * P, h0:h0 + rows].rearrange(
                        "c h w -> c (h w)"),
                    in_=ot[:, osl],
                )
        del st[b]

    # ---- emission ----
    load(0)

    ident = consts.tile([P, P], F32)
    make_identity(nc, ident)
    w1t_sb = consts.tile([P, CB, 4 * MID], BF16)
    nc.gpsimd.dma_start(out=w1t_sb, in_=w1t.rearrange("(cb c) m -> c cb m", cb=CB))
    w2r_sb = consts.tile([P, 9, P], BF16)
    nc.gpsimd.dma_start(out=w2r_sb, in_=w2r[:, :, :])
    weights = (w1t_sb, w2r_sb, ident)

    prep(0, weights)
    load(1)
    for b in range(BPC):
        if b + 2 < BPC:
            load(b + 2)
        if b + 1 < BPC:
            prep(b + 1, weights)
        conv(b)


def build_nc():
    nc = bass.Bass(trn_type="TRN2")
    x = nc.dram_tensor("x", [BPC, C, H, W], BF16, kind="ExternalInput")
    w1t = nc.dram_tensor("w1t", [C, 4 * MID], BF16, kind="ExternalInput")
    w2r = nc.dram_tensor("w2r", [P, 9, P], BF16, kind="ExternalInput")
    out = nc.dram_tensor("out", [BPC, C, H, W], F16, kind="ExternalOutput")
    with tile.TileContext(nc) as tc:
        _build_body(tc, x, w1t, w2r, out)
    return nc


def host_prep(w1: np.ndarray, w2: np.ndarray):
    """Layout/dtype-only prep of the (tiny) shared weights."""
    import ml_dtypes

    w1t = np.ascontiguousarray(np.asarray(w1, dtype=np.float32).T)  # [C, MID]
    w1t4 = np.tile(w1t, (1, 4))  # [C, 4*MID], replicated for row groups
    # w2 rows are r = c*9 + t ; -> [cb, t, mid, c_local], pre-scaled by 0.5
    w2r = np.asarray(w2, dtype=np.float32).reshape(CB, P, 9, MID)
    w2r = np.ascontiguousarray(w2r.transpose(0, 2, 3, 1)) * 0.5  # [cb,t,mid,c]
    w2r2 = np.zeros((P, 9, P), dtype=np.float32)
    for t in range(9):
        w2r2[0:MID, t, :] = w2r[0, t]
        w2r2[64:64 + MID, t, :] = w2r[1, t]
    return w1t4.astype(ml_dtypes.bfloat16), w2r2.astype(ml_dtypes.bfloat16)


# TPB instructions have a single EVENTS (wait) slot; move excess waits onto
# EventSemaphore instructions inserted just before (same engine queue).
_SPLIT_WAIT_SKIP = {"EventSemaphore"}


def _split_matmul_waits_json(data: bytes) -> bytes:
    import orjson

    m = orjson.loads(data)
    cnt = 0
    for fn in m.get("functions", []):
        for bb in fn.get("blocks", []):
            insts = bb.get("instructions")
            if not insts:
                continue
            out = []
            changed = False
            for ins in insts:
                si = ins.get("sync_info")
                if (
                    ins.get("opcode") not in _SPLIT_WAIT_SKIP
                    and si
                    and len(si.get("on_wait") or []) > 1
                ):
                    waits = si["on_wait"]
                    for w in waits[:-1]:
                        out.append({
                            "name": f"EVW-{cnt}",
                            "opcode": "EventSemaphore",
                            "engine": ins["engine"],
                            "ins": [],
                            "outs": [],
                            "debug": ins.get("debug", 0),
                            "sync_info": {"on_wait": [w], "on_update": []},
                        })
                        cnt += 1
                    si["on_wait"] = [waits[-1]]
                    changed = True
                out.append(ins)
            if changed:
                bb["instructions"] = out
    return orjson.dumps(m)


_CACHE: dict = {}


def _get_nc():
    if "nc" not in _CACHE:
        nc = build_nc()
        orig = nc.to_json_bytes
        nc.to_json_bytes = lambda: _split_matmul_waits_json(orig())
        _CACHE["nc"] = nc
    return _CACHE["nc"]


def kernel(x, w1, w2, trace: bool = False, **run_kwargs):
    import ml_dtypes
    x = np.ascontiguousarray(
        np.asarray(x, dtype=np.float32)).astype(ml_dtypes.bfloat16)
    assert x.shape == (B, C, H, W)
    w1t, w2r = host_prep(w1, w2)

    nc = _get_nc()
    in_maps = [
        {"x": x[i * BPC:(i + 1) * BPC], "w1t": w1t, "w2r": w2r}
        for i in range(NCORES)
    ]
    res = run_bass_kernel_spmd(
        nc, in_maps, core_ids=list(range(NCORES)), trace=trace, **run_kwargs
    )
    _CACHE["last_results"] = res
    out = np.concatenate(
        [np.asarray(res.results[i]["out"]) for i in range(NCORES)], axis=0
    ).astype(np.float32)
    return out


# revision 8
# speedup vs baseline: 1.1529x; 1.1529x over previous
"""Trainium2 Bass kernel for nn_DynamicSparseConv.

Model (per sample):
    y  = mean(x, HW)                        [C]
    h  = gelu(y @ w1.T)                     [MID]
    w  = softmax((h @ w2.T).reshape(C, 9))  per-channel 3x3 kernels
    out = depthwise3x3(x, w) + x

Sharding: pure data parallel, batch 32 -> 4 samples on each of 8 cores.

Per-core design (per (sample b, channel-block cb of 128)):
  - x arrives f32 [128, 4096]; one ACT pass per half does the f32->bf16
    cast INTO a zero-padded pitch-66 tile xc ([66 rows x 66 cols], one
    zero col on each side of every row, zero rows top/bottom) AND the
    channel sums via accum_out.  The pitch-66 layout makes every 3x3 tap
    a clean 2D view [[66,nr],[1,64]] -- no horizontal wrap-around
    garbage, so no edge fixups at all.
  - tiny MLP on PE (wgen), tanh-gelu, softmax: as before.
  - conv: 6 taps on PE as diagonal-weighted matmuls into PSUM (center
    tap's diag holds w_center+1, fusing the residual); ACT drains each
    PSUM chunk to the f16 output tile; DVE applies the remaining 3 taps
    as TS-mul(4x) + TT-add(2x) pairs on pitch-66 views.  The last
    sample runs all 9 taps on PE so the kernel tail is just drain+DMA.
  - output is written f16 (half the DMA bytes); the host upcasts.
"""

import numpy as np
from contextlib import ExitStack

import concourse.bass as bass
import concourse.tile as tile
from concourse import mybir
from concourse._compat import with_exitstack
from concourse.masks import make_identity
from concourse.bass_utils import run_bass_kernel_spmd

F32 = mybir.dt.float32
F16 = mybir.dt.float16
BF16 = mybir.dt.bfloat16
AL = mybir.AluOpType
AF = mybir.ActivationFunctionType

B, C, H, W = 32, 256, 64, 64
MID = 32
NCORES = 8
BPC = B // NCORES          # samples per core
P = 128
CB = C // P                # channel blocks
FREE = H * W               # 4096
NCHUNK = 8                 # PSUM chunks per tile (8 output rows each)
RPC = H // NCHUNK          # rows per chunk
CH = RPC * W               # 512 elements per PSUM chunk

PITCH = W + 2              # 66
XC = PITCH * (H + 2)       # 4356: rows -1..64, cols -1..64

# tap -> smw column
def _tcol(r, s):
    return (r + 1) * 3 + (s + 1)

# taps: center rides PE with weight w+1 (residual fused)
PE_TAPS = [(0, 0), (-1, -1), (-1, 1), (1, -1), (1, 1), (0, -1)]
DVE_TAPS = [(-1, 0), (1, 0), (0, 1)]

SQRT_2_OVER_PI = 0.7978845608028654
GELU_C = 0.044715


def _v66(t, row0, col, nr, inner=W):
    """2D view of the pitch-66 tile: rows row0..row0+nr-1 (x row index,
    -1-based pad space), starting at x col `col` (-1..64)."""
    return bass.AP(
        tensor=t.tensor,
        offset=t.offset + (row0 + 1) * PITCH + (col + 1),
        ap=[list(t.ap[0]), [PITCH, nr], [1, inner]],
    )


@with_exitstack
def _build_body(ctx: ExitStack, tc: "tile.TileContext", x, w1t, w2r, out):
    nc = tc.nc

    consts = ctx.enter_context(tc.tile_pool(name="consts", bufs=1))
    xspool = ctx.enter_context(tc.tile_pool(name="xspool", bufs=4))
    xcpool = ctx.enter_context(tc.tile_pool(name="xcpool", bufs=5))
    opool = ctx.enter_context(tc.tile_pool(name="opool", bufs=4))
    tpool = ctx.enter_context(tc.tile_pool(name="tpool", bufs=3))
    mpool = ctx.enter_context(tc.tile_pool(name="mpool", bufs=4))
    dpool = ctx.enter_context(tc.tile_pool(name="dpool", bufs=4 * 10))
    cpsum = ctx.enter_context(tc.tile_pool(name="cpsum", bufs=6, space="PSUM"))
    spsum = ctx.enter_context(tc.tile_pool(name="spsum", bufs=2, space="PSUM"))

    NSPLIT = 2
    HROWS = H // NSPLIT
    st = {}

    def load(b):
        """DMA f32 halves; gp memsets zero the pad cells of xc."""
        xss, xcs, dmas = [None] * CB, [None] * CB, []
        sums = mpool.tile([P, CB * NSPLIT], F32, name=f"sums{b}", tag="sums")
        for cb in ([1, 0] if b == 0 else range(CB)):
            xs = xspool.tile([P, FREE], BF16, name=f"xs{b}_{cb}", tag="xs")
            xc = xcpool.tile([P, XC], BF16, name=f"xc{b}_{cb}", tag="xc")
            # pads: top row (-1), bottom row (64), and the col-pad pairs
            # (col 64 of row r, col -1 of row r+1) which are adjacent.
            nc.gpsimd.memset(xc[:, 0:PITCH], 0.0)
            nc.gpsimd.memset(xc[:, XC - PITCH:XC], 0.0)
            pairs = bass.AP(
                tensor=xc.tensor, offset=xc.offset + PITCH - 1,
                ap=[list(xc.ap[0]), [PITCH, H + 1], [1, 2]],
            )
            nc.gpsimd.memset(pairs, 0.0)
            xsrc = x[b, cb * P:(cb + 1) * P].rearrange("c h w -> c (h w)")
            for j in range(NSPLIT):
                sl = slice(j * HROWS * W, (j + 1) * HROWS * W)
                dmas.append(nc.sync.dma_start(out=xs[:, sl], in_=xsrc[:, sl]))
                scol = sums[:, cb * NSPLIT + j:cb * NSPLIT + j + 1]
                if b == 0 and cb == 0:
                    # head: ACT is choked on casts; DVE (idle) computes the
                    # sums in parallel, the cast runs without accum
                    nc.vector.tensor_reduce(
                        out=scol, in_=xs[:, sl], op=AL.add,
                        axis=mybir.AxisListType.X)
                    nc.scalar.activation(
                        out=_v66(xc, j * HROWS, 0, HROWS),
                        in_=xs[:, sl], func=AF.Copy)
                else:
                    nc.scalar.activation(
                        out=_v66(xc, j * HROWS, 0, HROWS),
                        in_=xs[:, sl], func=AF.Copy, accum_out=scol)
            xss[cb] = xs
            xcs[cb] = xc
        st[b] = {"xss": xss, "xcs": xcs, "sums": sums, "dmas": dmas}

    def prep(b, weights):
        """MLP -> softmax tap weights -> diagonal matrices for sample b."""
        w1t_sb, w2r_sb, ident = weights
        sums = st[b]["sums"]
        ncols = CB * NSPLIT
        sums_bf = mpool.tile([P, ncols], BF16, name=f"sums_bf{b}", tag="sums_bf")
        # fold the 1/(H*W) of the mean in here
        nc.vector.tensor_scalar_mul(sums_bf, sums, 1.0 / FREE)
        hps = spsum.tile([P, 9], F32, name=f"hps{b}", tag="sps")
        for j in range(ncols):
            nc.tensor.matmul(
                hps[:, 0:1], lhsT=w1t_sb[:, j // NSPLIT, :],
                rhs=sums_bf[:, j:j + 1],
                start=(j == 0), stop=(j == ncols - 1),
            )
        # tanh-gelu (exact to ~1e-9 for |u|<0.05); chain stays on DVE except
        # the tanh itself so there are few cross-engine hops and no ACT
        # table swaps (Tanh+Exp share a table set, Gelu does not).
        u = mpool.tile([P, 1], F32, name=f"u{b}", tag="u")
        nc.vector.tensor_copy(u, hps[:, 0:1])
        sq = mpool.tile([P, 1], F32, name=f"sq{b}", tag="sq")
        nc.vector.tensor_mul(sq, u, u)
        c1 = mpool.tile([P, 1], F32, name=f"c1{b}", tag="c1")
        nc.vector.tensor_scalar(
            out=c1, in0=sq, scalar1=GELU_C, scalar2=1.0, op0=AL.mult, op1=AL.add,
        )
        arg = mpool.tile([P, 1], F32, name=f"arg{b}", tag="arg")
        nc.vector.tensor_mul(arg, u, c1)
        th = mpool.tile([P, 1], F32, name=f"th{b}", tag="th")
        nc.scalar.activation(th, arg, AF.Tanh, scale=SQRT_2_OVER_PI)
        g4 = mpool.tile([P, 1], BF16, name=f"g4{b}", tag="g4")
        nc.vector.tensor_scalar(
            out=g4, in0=th, scalar1=1.0, scalar2=u, op0=AL.add, op1=AL.mult,
        )

        # 18 wgen matmuls in the two 64-row PE tile groups
        wgs = [spsum.tile([P, 9], F32, name=f"wg{b}_{cb}", tag="sps")
               for cb in range(CB)]
        for t in range(9):
            for cb in range(CB):
                rg = 64 * cb
                nc.tensor.matmul(
                    wgs[cb][:, t:t + 1],
                    lhsT=w2r_sb[rg:rg + MID, t, :],
                    rhs=g4[rg:rg + MID, 0:1],
                    start=True, stop=True,
                    tile_position=(rg, 0),
                )

        st[b]["smw"] = [None] * CB
        st[b]["diags"] = [None] * CB
        last = b == BPC - 1
        cbs = [1, 0] if b == 0 else list(range(CB))
        for cb in cbs:
            pe_taps = PE_TAPS + (DVE_TAPS if (last and cb == CB - 1) else [])
            ew = mpool.tile([P, 9], F32, name=f"ew{b}_{cb}", tag="ew")
            den = mpool.tile([P, 1], F32, name=f"den{b}_{cb}", tag="den")
            nc.scalar.activation(ew, wgs[cb], AF.Exp, accum_out=den)
            rden = mpool.tile([P, 1], F32, name=f"rden{b}_{cb}", tag="rden")
            nc.vector.reciprocal(rden, den)
            smw = mpool.tile([P, 9], F32, name=f"smw{b}_{cb}", tag="smw")
            nc.vector.tensor_scalar_mul(smw, ew, rden)
            # center diag coefficient: w_center + 1 (residual fused)
            wc1 = mpool.tile([P, 1], F32, name=f"wc1{b}_{cb}", tag="wc1")
            nc.vector.tensor_scalar_add(wc1, smw[:, 4:5], 1.0)

            diags = {}
            for k, (r, s) in enumerate(pe_taps):
                tcol = _tcol(r, s)
                dg = dpool.tile([P, P], BF16, name=f"dg{b}_{cb}_{tcol}", tag="dg")
                sca = wc1 if (r, s) == (0, 0) else smw[:, tcol:tcol + 1]
                # sample 0: all diags on DVE (ACT is the head serializer);
                # later samples alternate ACT/DVE
                if b > 0 and k % 2 == 0:
                    nc.scalar.mul(dg, ident, sca)
                else:
                    nc.vector.tensor_scalar_mul(dg, ident, sca)
                diags[(r, s)] = dg
            st[b]["smw"][cb] = smw
            st[b]["diags"][cb] = diags

    def conv(b):
        """Depthwise conv: PE taps -> PSUM, ACT drain -> f16 ot, DVE taps
        accumulate on pitch-66 views, then DMA out per half."""
        last = b == BPC - 1
        cbs = [1, 0] if b == 0 else list(range(CB))
        for cb in cbs:
            pe_taps = PE_TAPS + (DVE_TAPS if (last and cb == CB - 1) else [])
            xc = st[b]["xcs"][cb]
            smw = st[b]["smw"][cb]
            diags = st[b]["diags"][cb]
            ot = opool.tile([P, FREE], F16, name=f"ot{b}_{cb}", tag="ot")
            for q in range(NCHUNK):
                ps = cpsum.tile([P, CH], F32, name=f"ps{b}_{cb}_{q}", tag="ps")
                for i, (r, s) in enumerate(pe_taps):
                    nc.tensor.matmul(
                        ps,
                        lhsT=diags[(r, s)],
                        rhs=_v66(xc, q * RPC + r, s, RPC),
                        start=(i == 0), stop=(i == len(pe_taps) - 1),
                    )
                # drain PSUM chunk into the f16 output tile: DVE early in
                # the pipeline (ACT is cast-choked there), ACT later (DVE
                # becomes the busier engine in steady state)
                if (b == 0 and cb == 0) or q == NCHUNK - 1:
                    nc.vector.tensor_copy(ot[:, q * CH:(q + 1) * CH], ps)
                else:
                    nc.scalar.activation(
                        out=ot[:, q * CH:(q + 1) * CH], in_=ps, func=AF.Copy)

            # finer pieces late in the pipeline shorten the serial tail
            npiece = 4 if (last and cb == 0) else 2
            rows = H // npiece
            for piece in range(npiece):
                h0 = piece * rows
                osl = slice(h0 * W, (h0 + rows) * W)
                if not (last and cb == CB - 1):
                    tmp = tpool.tile([P, rows * W], BF16,
                                     name=f"tm{b}_{cb}_{piece}", tag="tm")
                    for (r, s) in DVE_TAPS:
                        tcol = _tcol(r, s)
                        nc.vector.tensor_scalar_mul(
                            tmp, _v66(xc, h0 + r, s, rows),
                            smw[:, tcol:tcol + 1])
                        nc.vector.tensor_tensor(
                            out=ot[:, osl], in0=ot[:, osl], in1=tmp, op=AL.add)
                eng = nc.sync if (cb + piece) % 2 == 0 else nc.gpsimd
                eng.dma_start(
                    out=out[b, cb * P:(cb + 1) * P, h0:h0 + rows].rearrange(
                        "c h w -> c (h w)"),
                    in_=ot[:, osl],
                )
        del st[b]

    # ---- emission ----
    load(0)

    ident = consts.tile([P, P], F32)
    make_identity(nc, ident)
    w1t_sb = consts.tile([P, CB, 4 * MID], BF16)
    nc.gpsimd.dma_start(out=w1t_sb, in_=w1t.rearrange("(cb c) m -> c cb m", cb=CB))
    w2r_sb = consts.tile([P, 9, P], BF16)
    nc.gpsimd.dma_start(out=w2r_sb, in_=w2r[:, :, :])
    weights = (w1t_sb, w2r_sb, ident)

    prep(0, weights)
    load(1)
    for b in range(BPC):
        if b + 2 < BPC:
            load(b + 2)
        if b + 1 < BPC:
            prep(b + 1, weights)
        conv(b)


def build_nc():
    nc = bass.Bass(trn_type="TRN2")
    x = nc.dram_tensor("x", [BPC, C, H, W], BF16, kind="ExternalInput")
    w1t = nc.dram_tensor("w1t", [C, 4 * MID], BF16, kind="ExternalInput")
    w2r = nc.dram_tensor("w2r", [P, 9, P], BF16, kind="ExternalInput")
    out = nc.dram_tensor("out", [BPC, C, H, W], F16, kind="ExternalOutput")
    with tile.TileContext(nc) as tc:
        _build_body(tc, x, w1t, w2r, out)
    return nc


def host_prep(w1: np.ndarray, w2: np.ndarray):
    """Layout/dtype-only prep of the (tiny) shared weights."""
    import ml_dtypes

    w1t = np.ascontiguousarray(np.asarray(w1, dtype=np.float32).T)  # [C, MID]
    w1t4 = np.tile(w1t, (1, 4))  # [C, 4*MID], replicated for row groups
    # w2 rows are r = c*9 + t ; -> [cb, t, mid, c_local], pre-scaled by 0.5
    w2r = np.asarray(w2, dtype=np.float32).reshape(CB, P, 9, MID)
    w2r = np.ascontiguousarray(w2r.transpose(0, 2, 3, 1)) * 0.5  # [cb,t,mid,c]
    w2r2 = np.zeros((P, 9, P), dtype=np.float32)
    for t in range(9):
        w2r2[0:MID, t, :] = w2r[0, t]
        w2r2[64:64 + MID, t, :] = w2r[1, t]
    return w1t4.astype(ml_dtypes.bfloat16), w2r2.astype(ml_dtypes.bfloat16)


# TPB instructions have a single EVENTS (wait) slot; move excess waits onto
# EventSemaphore instructions inserted just before (same engine queue).
_SPLIT_WAIT_SKIP = {"EventSemaphore"}


def _split_matmul_waits_json(data: bytes) -> bytes:
    import orjson

    m = orjson.loads(data)
    cnt = 0
    for fn in m.get("functions", []):
        for bb in fn.get("blocks", []):
            insts = bb.get("instructions")
            if not insts:
                continue
            out = []
            changed = False
            for ins in insts:
                si = ins.get("sync_info")
                if (
                    ins.get("opcode") not in _SPLIT_WAIT_SKIP
                    and si
                    and len(si.get("on_wait") or []) > 1
                ):
                    waits = si["on_wait"]
                    for w in waits[:-1]:
                        out.append({
                            "name": f"EVW-{cnt}",
                            "opcode": "EventSemaphore",
                            "engine": ins["engine"],
                            "ins": [],
                            "outs": [],
                            "debug": ins.get("debug", 0),
                            "sync_info": {"on_wait": [w], "on_update": []},
                        })
                        cnt += 1
                    si["on_wait"] = [waits[-1]]
                    changed = True
                out.append(ins)
            if changed:
                bb["instructions"] = out
    return orjson.dumps(m)


_CACHE: dict = {}


def _get_nc():
    if "nc" not in _CACHE:
        nc = build_nc()
        orig = nc.to_json_bytes
        nc.to_json_bytes = lambda: _split_matmul_waits_json(orig())
        _CACHE["nc"] = nc
    return _CACHE["nc"]


def kernel(x, w1, w2, trace: bool = False, **run_kwargs):
    import ml_dtypes
    x = np.ascontiguousarray(
        np.asarray(x, dtype=np.float32)).astype(ml_dtypes.bfloat16)
    assert x.shape == (B, C, H, W)
    w1t, w2r = host_prep(w1, w2)

    nc = _get_nc()
    in_maps = [
        {"x": x[i * BPC:(i + 1) * BPC], "w1t": w1t, "w2r": w2r}
        for i in range(NCORES)
    ]
    res = run_bass_kernel_spmd(
        nc, in_maps, core_ids=list(range(NCORES)), trace=trace, **run_kwargs
    )
    _CACHE["last_results"] = res
    out = np.concatenate(
        [np.asarray(res.results[i]["out"]) for i in range(NCORES)], axis=0
    ).astype(np.float32)
    return out


# revision 9
# speedup vs baseline: 1.2038x; 1.0441x over previous
"""Trainium2 Bass kernel for nn_DynamicSparseConv.

Model (per sample):
    y  = mean(x, HW)                        [C]
    h  = gelu(y @ w1.T)                     [MID]
    w  = softmax((h @ w2.T).reshape(C, 9))  per-channel 3x3 kernels
    out = depthwise3x3(x, w) + x

Sharding: pure data parallel, batch 32 -> 4 samples on each of 8 cores.

Per-core design (per (sample b, channel-block cb of 128)):
  - x arrives f32 [128, 4096]; one ACT pass per half does the f32->bf16
    cast INTO a zero-padded pitch-66 tile xc ([66 rows x 66 cols], one
    zero col on each side of every row, zero rows top/bottom) AND the
    channel sums via accum_out.  The pitch-66 layout makes every 3x3 tap
    a clean 2D view [[66,nr],[1,64]] -- no horizontal wrap-around
    garbage, so no edge fixups at all.
  - tiny MLP on PE (wgen), tanh-gelu, softmax: as before.
  - conv: 6 taps on PE as diagonal-weighted matmuls into PSUM (center
    tap's diag holds w_center+1, fusing the residual); ACT drains each
    PSUM chunk to the f16 output tile; DVE applies the remaining 3 taps
    as TS-mul(4x) + TT-add(2x) pairs on pitch-66 views.  The last
    sample runs all 9 taps on PE so the kernel tail is just drain+DMA.
  - output is written f16 (half the DMA bytes); the host upcasts.
"""

import numpy as np
from contextlib import ExitStack

import concourse.bass as bass
import concourse.tile as tile
from concourse import mybir
from concourse._compat import with_exitstack
from concourse.masks import make_identity
from concourse.bass_utils import run_bass_kernel_spmd

F32 = mybir.dt.float32
F16 = mybir.dt.float16
BF16 = mybir.dt.bfloat16
AL = mybir.AluOpType
AF = mybir.ActivationFunctionType

B, C, H, W = 32, 256, 64, 64
MID = 32
NCORES = 8
BPC = B // NCORES          # samples per core
P = 128
CB = C // P                # channel blocks
FREE = H * W               # 4096
NCHUNK = 8                 # PSUM chunks per tile (8 output rows each)
RPC = H // NCHUNK          # rows per chunk
CH = RPC * W               # 512 elements per PSUM chunk

PITCH = W + 2              # 66
XC = PITCH * (H + 2)       # 4356: rows -1..64, cols -1..64

# tap -> smw column
def _tcol(r, s):
    return (r + 1) * 3 + (s + 1)

# taps: center rides PE with weight w+1 (residual fused)
PE_TAPS = [(0, 0), (-1, -1), (-1, 1), (1, -1), (1, 1), (0, -1)]
DVE_TAPS = [(-1, 0), (1, 0), (0, 1)]

SQRT_2_OVER_PI = 0.7978845608028654
GELU_C = 0.044715


def _v66(t, row0, col, nr, inner=W):
    """2D view of the pitch-66 tile: rows row0..row0+nr-1 (x row index,
    -1-based pad space), starting at x col `col` (-1..64)."""
    return bass.AP(
        tensor=t.tensor,
        offset=t.offset + (row0 + 1) * PITCH + (col + 1),
        ap=[list(t.ap[0]), [PITCH, nr], [1, inner]],
    )


@with_exitstack
def _build_body(ctx: ExitStack, tc: "tile.TileContext", x, w1t, w2r, out):
    nc = tc.nc

    consts = ctx.enter_context(tc.tile_pool(name="consts", bufs=1))
    xspool = ctx.enter_context(tc.tile_pool(name="xspool", bufs=4))
    xcpool = ctx.enter_context(tc.tile_pool(name="xcpool", bufs=5))
    opool = ctx.enter_context(tc.tile_pool(name="opool", bufs=4))
    tpool = ctx.enter_context(tc.tile_pool(name="tpool", bufs=3))
    mpool = ctx.enter_context(tc.tile_pool(name="mpool", bufs=4))
    dpool = ctx.enter_context(tc.tile_pool(name="dpool", bufs=4 * 10))
    cpsum = ctx.enter_context(tc.tile_pool(name="cpsum", bufs=6, space="PSUM"))
    spsum = ctx.enter_context(tc.tile_pool(name="spsum", bufs=2, space="PSUM"))

    NSPLIT = 2
    HROWS = H // NSPLIT
    st = {}

    def load(b):
        """DMA f32 halves; gp memsets zero the pad cells of xc."""
        xss, xcs, dmas = [None] * CB, [None] * CB, []
        sums = mpool.tile([P, CB * NSPLIT], F32, name=f"sums{b}", tag="sums")
        for cb in range(CB):
            xs = xspool.tile([P, FREE], BF16, name=f"xs{b}_{cb}", tag="xs")
            xc = xcpool.tile([P, XC], BF16, name=f"xc{b}_{cb}", tag="xc")
            # pads: top row (-1), bottom row (64), and the col-pad pairs
            # (col 64 of row r, col -1 of row r+1) which are adjacent.
            nc.gpsimd.memset(xc[:, 0:PITCH], 0.0)
            nc.gpsimd.memset(xc[:, XC - PITCH:XC], 0.0)
            pairs = bass.AP(
                tensor=xc.tensor, offset=xc.offset + PITCH - 1,
                ap=[list(xc.ap[0]), [PITCH, H + 1], [1, 2]],
            )
            nc.gpsimd.memset(pairs, 0.0)
            xsrc = x[b, cb * P:(cb + 1) * P].rearrange("c h w -> c (h w)")
            for j in range(NSPLIT):
                sl = slice(j * HROWS * W, (j + 1) * HROWS * W)
                dmas.append(nc.sync.dma_start(out=xs[:, sl], in_=xsrc[:, sl]))
                # one ACT pass: bf16 repitch into pitch-66 AND partial sums
                nc.scalar.activation(
                    out=_v66(xc, j * HROWS, 0, HROWS),
                    in_=xs[:, sl], func=AF.Copy,
                    accum_out=sums[:, cb * NSPLIT + j:cb * NSPLIT + j + 1])
            xss[cb] = xs
            xcs[cb] = xc
        st[b] = {"xss": xss, "xcs": xcs, "sums": sums, "dmas": dmas}

    def prep(b, weights):
        """MLP -> softmax tap weights -> diagonal matrices for sample b."""
        w1t_sb, w2r_sb, ident = weights
        sums = st[b]["sums"]
        ncols = CB * NSPLIT
        sums_bf = mpool.tile([P, ncols], BF16, name=f"sums_bf{b}", tag="sums_bf")
        # fold the 1/(H*W) of the mean in here
        nc.vector.tensor_scalar_mul(sums_bf, sums, 1.0 / FREE)
        hps = spsum.tile([P, 9], F32, name=f"hps{b}", tag="sps")
        for j in range(ncols):
            nc.tensor.matmul(
                hps[:, 0:1], lhsT=w1t_sb[:, j // NSPLIT, :],
                rhs=sums_bf[:, j:j + 1],
                start=(j == 0), stop=(j == ncols - 1),
            )
        # tanh-gelu (exact to ~1e-9 for |u|<0.05); chain stays on DVE except
        # the tanh itself so there are few cross-engine hops and no ACT
        # table swaps (Tanh+Exp share a table set, Gelu does not).
        u = mpool.tile([P, 1], F32, name=f"u{b}", tag="u")
        nc.vector.tensor_copy(u, hps[:, 0:1])
        sq = mpool.tile([P, 1], F32, name=f"sq{b}", tag="sq")
        nc.vector.tensor_mul(sq, u, u)
        c1 = mpool.tile([P, 1], F32, name=f"c1{b}", tag="c1")
        nc.vector.tensor_scalar(
            out=c1, in0=sq, scalar1=GELU_C, scalar2=1.0, op0=AL.mult, op1=AL.add,
        )
        arg = mpool.tile([P, 1], F32, name=f"arg{b}", tag="arg")
        nc.vector.tensor_mul(arg, u, c1)
        th = mpool.tile([P, 1], F32, name=f"th{b}", tag="th")
        nc.scalar.activation(th, arg, AF.Tanh, scale=SQRT_2_OVER_PI)
        g4 = mpool.tile([P, 1], BF16, name=f"g4{b}", tag="g4")
        nc.vector.tensor_scalar(
            out=g4, in0=th, scalar1=1.0, scalar2=u, op0=AL.add, op1=AL.mult,
        )

        # 18 wgen matmuls in the two 64-row PE tile groups
        wgs = [spsum.tile([P, 9], F32, name=f"wg{b}_{cb}", tag="sps")
               for cb in range(CB)]
        for t in range(9):
            for cb in range(CB):
                rg = 64 * cb
                nc.tensor.matmul(
                    wgs[cb][:, t:t + 1],
                    lhsT=w2r_sb[rg:rg + MID, t, :],
                    rhs=g4[rg:rg + MID, 0:1],
                    start=True, stop=True,
                    tile_position=(rg, 0),
                )

        st[b]["smw"] = [None] * CB
        st[b]["diags"] = [None] * CB
        last = b == BPC - 1
        for cb in range(CB):
            pe_taps = PE_TAPS + (DVE_TAPS if (last and cb == CB - 1) else [])
            ew = mpool.tile([P, 9], F32, name=f"ew{b}_{cb}", tag="ew")
            den = mpool.tile([P, 1], F32, name=f"den{b}_{cb}", tag="den")
            nc.scalar.activation(ew, wgs[cb], AF.Exp, accum_out=den)
            rden = mpool.tile([P, 1], F32, name=f"rden{b}_{cb}", tag="rden")
            nc.vector.reciprocal(rden, den)
            smw = mpool.tile([P, 9], F32, name=f"smw{b}_{cb}", tag="smw")
            nc.vector.tensor_scalar_mul(smw, ew, rden)
            # center diag coefficient: w_center + 1 (residual fused)
            wc1 = mpool.tile([P, 1], F32, name=f"wc1{b}_{cb}", tag="wc1")
            nc.vector.tensor_scalar_add(wc1, smw[:, 4:5], 1.0)

            diags = {}
            for k, (r, s) in enumerate(pe_taps):
                tcol = _tcol(r, s)
                dg = dpool.tile([P, P], BF16, name=f"dg{b}_{cb}_{tcol}", tag="dg")
                sca = wc1 if (r, s) == (0, 0) else smw[:, tcol:tcol + 1]
                # sample 0: all diags on DVE (ACT is the head serializer);
                # later samples alternate ACT/DVE
                if b > 0 and k % 2 == 0:
                    nc.scalar.mul(dg, ident, sca)
                else:
                    nc.vector.tensor_scalar_mul(dg, ident, sca)
                diags[(r, s)] = dg
            st[b]["smw"][cb] = smw
            st[b]["diags"][cb] = diags

    def conv(b):
        """Depthwise conv: PE taps -> PSUM, ACT drain -> f16 ot, DVE taps
        accumulate on pitch-66 views, then DMA out per half."""
        last = b == BPC - 1
        for cb in range(CB):
            pe_taps = PE_TAPS + (DVE_TAPS if (last and cb == CB - 1) else [])
            xc = st[b]["xcs"][cb]
            smw = st[b]["smw"][cb]
            diags = st[b]["diags"][cb]
            ot = opool.tile([P, FREE], F16, name=f"ot{b}_{cb}", tag="ot")
            for q in range(NCHUNK):
                ps = cpsum.tile([P, CH], F32, name=f"ps{b}_{cb}_{q}", tag="ps")
                for i, (r, s) in enumerate(pe_taps):
                    nc.tensor.matmul(
                        ps,
                        lhsT=diags[(r, s)],
                        rhs=_v66(xc, q * RPC + r, s, RPC),
                        start=(i == 0), stop=(i == len(pe_taps) - 1),
                    )
                # drain PSUM chunk into the f16 output tile: DVE early in
                # the pipeline (ACT is cast-choked there), ACT later (DVE
                # becomes the busier engine in steady state)
                if q == NCHUNK - 1:
                    nc.vector.tensor_copy(ot[:, q * CH:(q + 1) * CH], ps)
                else:
                    nc.scalar.activation(
                        out=ot[:, q * CH:(q + 1) * CH], in_=ps, func=AF.Copy)

            # finer pieces late in the pipeline shorten the serial tail
            npiece = 4 if (last and cb == 0) else 2
            rows = H // npiece
            for piece in range(npiece):
                h0 = piece * rows
                osl = slice(h0 * W, (h0 + rows) * W)
                if not (last and cb == CB - 1):
                    tmp = tpool.tile([P, rows * W], BF16,
                                     name=f"tm{b}_{cb}_{piece}", tag="tm")
                    for (r, s) in DVE_TAPS:
                        tcol = _tcol(r, s)
                        nc.vector.tensor_scalar_mul(
                            tmp, _v66(xc, h0 + r, s, rows),
                            smw[:, tcol:tcol + 1])
                        nc.vector.tensor_tensor(
                            out=ot[:, osl], in0=ot[:, osl], in1=tmp, op=AL.add)
                eng = nc.sync if (cb + piece) % 2 == 0 else nc.gpsimd
                eng.dma_start(
                    out=out[b, cb * P:(cb + 1) * P, h0:h0 + rows].rearrange(
                        "c h w -> c (h w)"),
                    in_=ot[:, osl],
                )
        del st[b]

    # ---- emission ----
    load(0)

    ident = consts.tile([P, P], F32)
    make_identity(nc, ident)
    w1t_sb = consts.tile([P, CB, 4 * MID], BF16)
    nc.gpsimd.dma_start(out=w1t_sb, in_=w1t.rearrange("(cb c) m -> c cb m", cb=CB))
    w2r_sb = consts.tile([P, 9, P], BF16)
    nc.gpsimd.dma_start(out=w2r_sb, in_=w2r[:, :, :])
    weights = (w1t_sb, w2r_sb, ident)

    prep(0, weights)
    load(1)
    for b in range(BPC):
        if b + 2 < BPC:
            load(b + 2)
        if b + 1 < BPC:
            prep(b + 1, weights)
        conv(b)


def build_nc():
    nc = bass.Bass(trn_type="TRN2")
    x = nc.dram_tensor("x", [BPC, C, H, W], BF16, kind="ExternalInput")
    w1t = nc.dram_tensor("w1t", [C, 4 * MID], BF16, kind="ExternalInput")
    w2r = nc.dram_tensor("w2r", [P, 9, P], BF16, kind="ExternalInput")
    out = nc.dram_tensor("out", [BPC, C, H, W], F16, kind="ExternalOutput")
    with tile.TileContext(nc) as tc:
        _build_body(tc, x, w1t, w2r, out)
    return nc


def host_prep(w1: np.ndarray, w2: np.ndarray):
    """Layout/dtype-only prep of the (tiny) shared weights."""
    import ml_dtypes

    w1t = np.ascontiguousarray(np.asarray(w1, dtype=np.float32).T)  # [C, MID]
    w1t4 = np.tile(w1t, (1, 4))  # [C, 4*MID], replicated for row groups
    # w2 rows are r = c*9 + t ; -> [cb, t, mid, c_local], pre-scaled by 0.5
    w2r = np.asarray(w2, dtype=np.float32).reshape(CB, P, 9, MID)
    w2r = np.ascontiguousarray(w2r.transpose(0, 2, 3, 1)) * 0.5  # [cb,t,mid,c]
    w2r2 = np.zeros((P, 9, P), dtype=np.float32)
    for t in range(9):
        w2r2[0:MID, t, :] = w2r[0, t]
        w2r2[64:64 + MID, t, :] = w2r[1, t]
    return w1t4.astype(ml_dtypes.bfloat16), w2r2.astype(ml_dtypes.bfloat16)


# TPB instructions have a single EVENTS (wait) slot; move excess waits onto
# EventSemaphore instructions inserted just before (same engine queue).
_SPLIT_WAIT_SKIP = {"EventSemaphore"}


def _split_matmul_waits_json(data: bytes) -> bytes:
    import orjson

    m = orjson.loads(data)
    cnt = 0
    for fn in m.get("functions", []):
        for bb in fn.get("blocks", []):
            insts = bb.get("instructions")
            if not insts:
                continue
            out = []
            changed = False
            for ins in insts:
                si = ins.get("sync_info")
                if (
                    ins.get("opcode") not in _SPLIT_WAIT_SKIP
                    and si
                    and len(si.get("on_wait") or []) > 1
                ):
                    waits = si["on_wait"]
                    for w in waits[:-1]:
                        out.append({
                            "name": f"EVW-{cnt}",
                            "opcode": "EventSemaphore",
                            "engine": ins["engine"],
                            "ins": [],
                            "outs": [],
                            "debug": ins.get("debug", 0),
                            "sync_info": {"on_wait": [w], "on_update": []},
                        })
                        cnt += 1
                    si["on_wait"] = [waits[-1]]
                    changed = True
                out.append(ins)
            if changed:
                bb["instructions"] = out
    return orjson.dumps(m)


_CACHE: dict = {}


def _get_nc():
    if "nc" not in _CACHE:
        nc = build_nc()
        orig = nc.to_json_bytes
        nc.to_json_bytes = lambda: _split_matmul_waits_json(orig())
        _CACHE["nc"] = nc
    return _CACHE["nc"]


def kernel(x, w1, w2, trace: bool = False, **run_kwargs):
    import ml_dtypes
    x = np.ascontiguousarray(
        np.asarray(x, dtype=np.float32)).astype(ml_dtypes.bfloat16)
    assert x.shape == (B, C, H, W)
    w1t, w2r = host_prep(w1, w2)

    nc = _get_nc()
    in_maps = [
        {"x": x[i * BPC:(i + 1) * BPC], "w1t": w1t, "w2r": w2r}
        for i in range(NCORES)
    ]
    res = run_bass_kernel_spmd(
        nc, in_maps, core_ids=list(range(NCORES)), trace=trace, **run_kwargs
    )
    _CACHE["last_results"] = res
    out = np.concatenate(
        [np.asarray(res.results[i]["out"]) for i in range(NCORES)], axis=0
    ).astype(np.float32)
    return out
